# revision 1
# baseline (speedup 1.0000x reference)
# Multi-head attention (b=4, s=1024, d=1024, h=16, hd=64, no mask) on 8
# Trainium2 NeuronCores via Bass/Tile.
#
# Sharding: core c = (bi, g) with bi = c // 2 (batch), g = c % 2 (head group
# of 8 heads = 512 of the 1024 projection output dims).  Each core computes
# a partial out-projection for its batch; the host sums the two partials per
# batch and adds the bias.
#
# Per-core dataflow (all device tensors pre-transposed on the host so the
# contraction dim always lands on SBUF partitions):
#   qT = WqT.T @ xT   -> [dj_local 512, si 1024]   (dj on partitions)
#   kT = WkT.T @ xT   -> same layout
#   v  = xT.T  @ WvT  -> [si 1024, dj_local 512]   (si on partitions)
#   per head h (64 dims), per kj-tile: scoresT[kj,qi] = kT_h slice (K=64,
#     partition base 0/64 so head pairs row-pack concurrently in the PE)
#   exp via ScalarE (scale=1/8) straight out of PSUM, rolling 4-slot buffer
#   ctxT[d,qi] (+rowsum row) = [v_h | 1].T @ expT, accumulated over kj,
#     software-pipelined one kj behind the scores/exp stream
#   normalize per head-pair: DVE reciprocal of the rowsum rows + a K=2
#     selector matmul broadcasts 1/rowsum across partitions + DVE multiply
#   out_partial[si,dj] = ctxT.T @ WoT  accumulated over the 512 local dims.
#
# All matmul operands are float32r (fp32 storage, bf16-pair PE decomposition:
# 1 cycle/row at N>=256 vs 4 for plain fp32).  Inputs are pre-rounded to the
# fp32r-representable set on the host (bf16 hi + bf16 lo split) so every
# device DMA is a plain byte copy (the BIR verifier requires fp32r matmul
# operands to be produced rounded).  Measured rel err vs fp64: 3.3e-4.

import sys

sys.path.insert(0, "/opt/trn_rl_repo")

import numpy as np

import concourse.bass as bass
import concourse.mybir as mybir
import concourse.tile as tile
from concourse import bacc
from concourse.bass_utils import run_bass_kernel_spmd

F32 = mybir.dt.float32
F32R = mybir.dt.float32r
BF16 = mybir.dt.bfloat16
EXP = mybir.ActivationFunctionType.Exp

P = 128          # SBUF partitions
B = 4            # batch
S = 1024         # sequence length
D = 1024         # model dim
DL = 512         # local head dims per core (8 heads x 64)
HD = 64          # head dim
NHL = 8          # heads per core
KT = D // P      # contraction tiles for projections (8)
SIT = S // P     # si tiles (8)
DJT = DL // P    # dj tiles / head pairs (4)
KJT = S // P     # kj tiles (8)
HALF = 512       # qi half width (PSUM-bank-sized matmul N)
NQH = S // HALF  # 2
SCALE = 1.0 / 8.0  # 1/sqrt(HD)

PROFILE = False          # set True (e.g. from test.py) to capture an NTFF trace
LAST_RESULTS = None      # BassKernelResults of the most recent run


def _build_program(phase="full"):
    # phase: cumulative prefix for timing attribution:
    #   "io" = input DMAs only, "proj" = +projections, "attn" = +attention,
    #   "norm" = +normalize, "full" = everything
    LV = {"io": 0, "proj": 1, "attn": 2, "norm": 3, "full": 4}[phase]
    nc = bacc.Bacc(
        "TRN2",
        target_bir_lowering=False,
        debug=False,
        enable_asserts=True,
        num_devices=8,
    )

    xT_d = nc.dram_tensor("xT", [D, S], F32R, kind="ExternalInput").ap()
    wq_d = nc.dram_tensor("wqT", [D, DL], F32R, kind="ExternalInput").ap()
    wk_d = nc.dram_tensor("wkT", [D, DL], F32R, kind="ExternalInput").ap()
    wv_d = nc.dram_tensor("wvT", [D, DL], F32R, kind="ExternalInput").ap()
    wo_d = nc.dram_tensor("woT", [DL, D], F32R, kind="ExternalInput").ap()
    out_d = nc.dram_tensor("out_p", [S, D], F32, kind="ExternalOutput").ap()

    with tile.TileContext(nc) as tc:
        with (
            nc.allow_low_precision(reason="fp32r matmul operands (rounded on store)"),
            tc.tile_pool(name="persist", bufs=1) as pers,
            tc.tile_pool(name="outsb", bufs=3) as osb,
            tc.tile_pool(name="psum", bufs=2, space="PSUM") as pp,
        ):
            # ---- persistent SBUF tensors -------------------------------
            qT_sb = pers.tile([P, DJT, S], F32R, name="qT_sb")
            kT_sb = pers.tile([P, DJT, S], F32R, name="kT_sb")
            # v staged with a ones column per head: [si-tile, head, 65]
            vst = pers.tile([P, SIT, NHL, HD + 1], F32R, name="vst")
            ctx_sb = pers.tile([P, DJT, S], F32R, name="ctx_sb")
            # rowsums: partition = head parity within pair, free = [pair, qi]
            rows_sb = pers.tile([2, DJT, S], F32R, name="rows_sb")
            sel_sb = pers.tile([2, P], F32R, name="sel_sb")

            # constants: pair-selector for the reciprocal broadcast matmul
            sel_np = np.zeros((2, P), dtype=np.float32)
            sel_np[0, 0:64] = 1.0
            sel_np[1, 64:128] = 1.0
            sel_dram = nc.inline_tensor(sel_np, name="sel_const")
            nc.sync.dma_start(sel_sb[:], sel_dram.ap().bitcast(F32R))
            ones_dram = nc.inline_tensor(
                np.ones((P, NHL), dtype=np.float32), name="ones_const"
            )
            for si in range(SIT):
                nc.sync.dma_start(
                    vst[:, si, :, HD : HD + 1],
                    ones_dram.ap().rearrange("p (h o) -> p h o", o=1).bitcast(F32R),
                )

            # ---- phase 1: projections ---------------------------------
            with tc.tile_pool(name="ph1", bufs=1) as ph1:
                xT_sb = ph1.tile([P, KT, S], F32R, name="xT_sb")
                wq_sb = ph1.tile([P, KT, DL], F32R, name="wq_sb")
                wk_sb = ph1.tile([P, KT, DL], F32R, name="wk_sb")
                wv_sb = ph1.tile([P, KT, DL], F32R, name="wv_sb")
                for i in range(KT):
                    # split xT tiles in half so a k-tile lands across queues
                    nc.sync.dma_start(
                        xT_sb[:, i, 0:HALF], xT_d[i * P : (i + 1) * P, 0:HALF]
                    )
                    nc.sync.dma_start(
                        xT_sb[:, i, HALF:S], xT_d[i * P : (i + 1) * P, HALF:S]
                    )
                    nc.sync.dma_start(wq_sb[:, i, :], wq_d[i * P : (i + 1) * P, :])
                for i in range(KT):
                    nc.sync.dma_start(wk_sb[:, i, :], wk_d[i * P : (i + 1) * P, :])
                for i in range(KT):
                    nc.sync.dma_start(wv_sb[:, i, :], wv_d[i * P : (i + 1) * P, :])

                # q/k: out[dj 128, si 512] = sum_k WT[k,dj].T @ xT[k,si]
                for w_sb, dest in ((wq_sb, qT_sb), (wk_sb, kT_sb)) if LV >= 1 else ():
                    for dj in range(DJT):
                        pst = pp.tile([P, S], F32, tag="pA", name="pst")
                        for k in range(KT):
                            for sh in range(NQH):
                                nc.tensor.matmul(
                                    pst[:, sh * HALF : (sh + 1) * HALF],
                                    (w_sb[:, k, dj * P : (dj + 1) * P]),
                                    (xT_sb[:, k, sh * HALF : (sh + 1) * HALF]),
                                    start=(k == 0),
                                    stop=(k == KT - 1),
                                )
                        nc.vector.tensor_copy(dest[:, dj, :], pst[:])

                # v: out[si 128, dj 512] = sum_k xT[k,si].T @ WvT[k,dj]
                for si in range(SIT if LV >= 1 else 0):
                    psv = pp.tile([P, DL], F32, tag="pA", name="psv")
                    for k in range(KT):
                        nc.tensor.matmul(
                            psv[:],
                            (xT_sb[:, k, si * P : (si + 1) * P]),
                            (wv_sb[:, k, :]),
                            start=(k == 0),
                            stop=(k == KT - 1),
                        )
                    nc.vector.tensor_copy(
                        vst[:, si, :, 0:HD],
                        psv.rearrange("p (h c) -> p h c", c=HD),
                    )

            # ---- phase 2: attention per (head, qi-half) ----------------
            with tc.tile_pool(name="ph2", bufs=3) as ph2:
                wo_sb = ph2.tile([P, DJT, D], F32R, name="wo_sb", tag="wo", bufs=1)
                for pi in range(DJT):
                    nc.sync.dma_start(wo_sb[:, pi, :], wo_d[pi * P : (pi + 1) * P, :])
                for h in range(NHL if LV >= 2 else 0):
                    pb = (h % 2) * 64  # partition base within the pair tile
                    pr = h // 2        # pair index
                    cx = pp.tile([HD + 1, S], F32, tag="pA", name="cx")
                    ets = {}

                    def _scores(kj, pb=pb, pr=pr, ets=ets):
                        sc = pp.tile([P, S], F32, tag="pB", name="sc")
                        for sh in range(NQH):
                            nc.tensor.matmul(
                                sc[:, sh * HALF : (sh + 1) * HALF],
                                (kT_sb[pb : pb + 64, pr, kj * P : (kj + 1) * P]),
                                (qT_sb[pb : pb + 64, pr, sh * HALF : (sh + 1) * HALF]),
                                start=True,
                                stop=True,
                            )
                        et = ph2.tile([P, S], F32R, tag="exp", name="et", bufs=4)
                        nc.scalar.activation(et[:], sc[:], EXP, scale=SCALE)
                        ets[kj] = et

                    def _ctx(kj, cx=cx, ets=ets):
                        et = ets.pop(kj)
                        for sh in range(NQH):
                            nc.tensor.matmul(
                                cx[:, sh * HALF : (sh + 1) * HALF],
                                (vst[:, kj, h, :]),
                                (et[:, sh * HALF : (sh + 1) * HALF]),
                                start=(kj == 0),
                                stop=(kj == KJT - 1),
                            )

                    _scores(0)
                    for kj in range(1, KJT):
                        _scores(kj)
                        _ctx(kj - 1)
                    _ctx(KJT - 1)
                    # PSUM can't be a DMA source: bounce via SBUF (DVE),
                    # then DMA shifts partitions into the pair layout.
                    cst = osb.tile([HD + 1, S], F32R, tag="cst", name="cst")
                    nc.vector.tensor_copy(cst[:], cx[:])
                    nc.sync.dma_start(
                        ctx_sb[pb : pb + 64, pr, :], cst[0:HD, :]
                    )
                    nc.sync.dma_start(
                        rows_sb[h % 2 : h % 2 + 1, pr, :], cst[HD : HD + 1, :]
                    )
                    # normalize the pair as soon as its odd head lands
                    if LV >= 3 and h % 2 == 1:
                        nc.vector.reciprocal(rows_sb[:, pr, :], rows_sb[:, pr, :])
                        rb = pp.tile([P, S], F32, tag="pA", name="rb")
                        for sh in range(NQH):
                            nc.tensor.matmul(
                                rb[:, sh * HALF : (sh + 1) * HALF],
                                (sel_sb[:]),
                                (rows_sb[:, pr, sh * HALF : (sh + 1) * HALF]),
                                start=True,
                                stop=True,
                            )
                        nc.vector.tensor_mul(
                            ctx_sb[:, pr, :], ctx_sb[:, pr, :], rb[:]
                        )

                # ---- phase 4: out-projection -------------------------
                if LV < 4:
                    # dummy output writer so the variant still has an output
                    dum = osb.tile([2, P], F32R, tag="cst", name="dum")
                    nc.vector.tensor_copy(dum[:], sel_sb[:])
                    nc.gpsimd.dma_start(out_d[0:2, 0:P], dum[:])
                for si in range(SIT if LV >= 4 else 0):
                    op = pp.tile([P, S], F32, tag="pB", name="op")
                    for pr in range(DJT):
                        for dh in range(NQH):
                            nc.tensor.matmul(
                                op[:, dh * HALF : (dh + 1) * HALF],
                                (ctx_sb[:, pr, si * P : (si + 1) * P]),
                                (wo_sb[:, pr, dh * HALF : (dh + 1) * HALF]),
                                start=(pr == 0),
                                stop=(pr == DJT - 1),
                            )
                    ot = osb.tile([P, S], F32, tag="ot", name="ot")
                    nc.vector.tensor_copy(ot[:], op[:])
                    for dh in range(NQH):
                        nc.sync.dma_start(
                            out_d[si * P : (si + 1) * P, dh * HALF : (dh + 1) * HALF],
                            ot[:, dh * HALF : (dh + 1) * HALF],
                        )

    nc.compile()
    return nc


_NC_CACHE = {}


def _get_program(phase="full"):
    if phase not in _NC_CACHE:
        _NC_CACHE[phase] = _build_program(phase)
    return _NC_CACHE[phase]


def _round_f32r(a):
    # fp32r = value representable as bf16_hi + bf16_lo; pre-round on host so
    # device DMAs are plain byte copies (verifier: producers must round)
    import ml_dtypes

    a = np.asarray(a, np.float32)
    hi = a.astype(ml_dtypes.bfloat16).astype(np.float32)
    lo = (a - hi).astype(ml_dtypes.bfloat16).astype(np.float32)
    return hi + lo


def make_in_maps(x, Wq, Wk, Wv, Wo):
    x = _round_f32r(x)
    Wq = _round_f32r(Wq)
    Wk = _round_f32r(Wk)
    Wv = _round_f32r(Wv)
    Wo = _round_f32r(Wo)
    in_maps = []
    for c in range(8):
        bi, g = divmod(c, 2)
        rs = slice(g * DL, (g + 1) * DL)
        in_maps.append(
            {
                "xT": np.ascontiguousarray(x[bi].T),
                "wqT": np.ascontiguousarray(Wq[rs, :].T),
                "wkT": np.ascontiguousarray(Wk[rs, :].T),
                "wvT": np.ascontiguousarray(Wv[rs, :].T),
                "woT": np.ascontiguousarray(Wo[:, rs].T),
            }
        )
    return in_maps


def kernel(x, Wq, Wk, Wv, Wo, bo):
    global LAST_RESULTS
    x = np.asarray(x, dtype=np.float32)
    Wq = np.asarray(Wq, dtype=np.float32)
    Wk = np.asarray(Wk, dtype=np.float32)
    Wv = np.asarray(Wv, dtype=np.float32)
    Wo = np.asarray(Wo, dtype=np.float32)
    bo = np.asarray(bo, dtype=np.float32)

    nc = _get_program()
    in_maps = make_in_maps(x, Wq, Wk, Wv, Wo)
    # retry once on transient device errors (e.g. NRT_EXEC_UNIT_UNRECOVERABLE
    # from a previous run wedging a core)
    import time as _time

    last_exc = None
    for attempt in range(3):
        try:
            res = run_bass_kernel_spmd(
                nc, in_maps, core_ids=list(range(8)), trace=PROFILE
            )
            break
        except Exception as e:  # noqa: BLE001
            last_exc = e
            if attempt == 2:
                raise
            _time.sleep(20)
    LAST_RESULTS = res
    parts = [r["out_p"] for r in res.results]
    out = np.empty((B, S, D), dtype=np.float32)
    for bi in range(B):
        out[bi] = parts[2 * bi] + parts[2 * bi + 1] + bo[None, :]
    return out

